# revision 4
# baseline (speedup 1.0000x reference)
"""GNN message passing (3 rounds) on 8 TRN2 NeuronCores — Bass/Tile kernel.

Linearity decomposition of the reference per round k:
  agg_v = deg_v*(Us_k@h_v) + sum_{e:src=v} (Ud_k@h_dst_e) + Ue_k@E_agg_v + deg_v*U_b_k
  h'_v  = relu(M1_k@h_v + M2_k@agg_v + M_b_k)
E_agg = segment_sum(edge_feat, src) is round-independent (precomputed once on device).

Sharding: contiguous src ranges of R=6272 nodes per core (graph parallel).  Per round,
cores AllGather T_k = h @ Ud_k.T (bf16 row table), gather rows by dst (dma_gather,
table split at 32768 for the int16 index limit), and segment-sum by src using one-hot
S-matrices on TensorE (edges pre-sorted by src; 64-node windows, 8 windows per PSUM
bank).  Dense transforms run as float32r matmuls with weights stationary.
"""

import numpy as np
import ml_dtypes

import concourse.bass as bass
import concourse.bacc as bacc
import concourse.tile as tile
import concourse.mybir as mybir
from concourse.bass_utils import run_bass_kernel_spmd
from concourse.masks import make_identity

NC = 8
N_NODES = 50000
D = 128
DE = 64
T = 3
SPLIT = 32768
W = 64                  # nodes per window / psum column block
R = 6272                # nodes per core (98 windows of 64)
NW = R // W             # 98
WPT = 8                 # windows per psum tile
NT = NW // WPT + (1 if NW % WPT else 0)   # 13
NROWT = R // 128        # 49 row-tiles per core
BF16 = mybir.dt.bfloat16
F32 = mybir.dt.float32
F32R = mybir.dt.float32r


def kernel(node_feat, edge_feat, src, dst, U_w, U_b, M_w, M_b):
    node_feat = np.asarray(node_feat, np.float32)
    edge_feat = np.asarray(edge_feat, np.float32)
    src64 = np.asarray(src).astype(np.int64)
    dst64 = np.asarray(dst).astype(np.int64)
    U_w = np.asarray(U_w, np.float32)
    U_b = np.asarray(U_b, np.float32)
    M_w = np.asarray(M_w, np.float32)
    M_b = np.asarray(M_b, np.float32)

    # ------------- host shard construction (pure index plumbing) -------------
    order = np.argsort(src64, kind="stable")
    src_s = src64[order]
    dst_s = dst64[order]
    core_of = src_s // R

    percore = []
    C_LO = C_HI = 1
    for c in range(NC):
        sel = np.nonzero(core_of == c)[0]
        sl = src_s[sel] - c * R
        win = sl // W
        low = dst_s[sel] < SPLIT
        wins = []
        for w in range(NW):
            m = win == w
            glo = sel[m & low]
            ghi = sel[m & ~low]
            C_LO = max(C_LO, (len(glo) + 127) // 128)
            C_HI = max(C_HI, (len(ghi) + 127) // 128)
            wins.append((glo, ghi))
        percore.append(wins)
    assert C_LO <= 8 and C_HI <= 8, (C_LO, C_HI)
    CPW = C_LO + C_HI
    NCH = NW * CPW

    in_maps = []
    for c in range(NC):
        wins = percore[c]
        idx_lo = np.zeros((128, NW * C_LO * 8), np.int16)
        idx_hi = np.zeros((128, NW * C_HI * 8), np.int16)
        S = np.zeros((128, NCH * W), ml_dtypes.bfloat16)
        efp = np.zeros((128, NCH * DE), np.float32)
        for w in range(NW):
            for half, (gsel, idx_t, C, coff) in enumerate(
                ((wins[w][0], idx_lo, C_LO, 0), (wins[w][1], idx_hi, C_HI, C_LO))
            ):
                n = len(gsel)
                if n == 0:
                    continue
                ii = np.arange(n)
                p = ii % 128
                ch = ii // 128
                dvals = (dst_s[gsel] - (SPLIT if half else 0)).astype(np.int16)
                svals = (src_s[gsel] - c * R - w * W).astype(np.int64)
                col0 = w * C * 8
                for g8 in range(8):
                    idx_t[16 * g8 + ii % 16, col0 + ii // 16] = dvals
                cbase = w * CPW + coff
                S[p, (cbase + ch) * W + svals] = 1.0
                cols = (cbase + ch)[:, None] * DE + np.arange(DE)[None, :]
                efp[p[:, None], cols] = edge_feat[order[gsel]]
        deg = np.zeros(R, np.float32)
        sel = np.nonzero(core_of == c)[0]
        np.add.at(deg, src_s[sel] - c * R, 1.0)
        nfp = np.zeros((128, NROWT * 128), np.float32)
        lo, hi = c * R, min((c + 1) * R, N_NODES)
        nfr = np.zeros((R, D), np.float32)
        nfr[: hi - lo] = node_feat[lo:hi]
        for t in range(NROWT):
            nfp[:, t * 128:(t + 1) * 128] = nfr[t * 128:(t + 1) * 128, :]
        in_maps.append(
            dict(
                idx_lo=idx_lo, idx_hi=idx_hi, S=S, efp=efp,
                deg=np.ascontiguousarray(np.broadcast_to(deg.astype(ml_dtypes.bfloat16), (128, R))),
                nfp=nfp,
                UsT=np.ascontiguousarray(U_w[:, :, :D].transpose(0, 2, 1)),
                UdT=np.ascontiguousarray(U_w[:, :, D:2 * D].transpose(0, 2, 1)),
                UeT=np.ascontiguousarray(U_w[:, :, 2 * D:].transpose(0, 2, 1)),
                UbR=np.ascontiguousarray(U_b.reshape(T, 1, D)),
                M1T=np.ascontiguousarray(M_w[:, :, :D].transpose(0, 2, 1)),
                M2T=np.ascontiguousarray(M_w[:, :, D:].transpose(0, 2, 1)),
                Mb=np.ascontiguousarray(M_b.reshape(T, D, 1)),
            )
        )
    use_ub = bool(np.any(U_b))

    # ------------------------- device program (SPMD) -------------------------
    nc = bacc.Bacc("TRN2", target_bir_lowering=False, debug=False, num_devices=NC)
    t_idx_lo = nc.dram_tensor("idx_lo", [128, NW * C_LO * 8], mybir.dt.int16, kind="ExternalInput")
    t_idx_hi = nc.dram_tensor("idx_hi", [128, NW * C_HI * 8], mybir.dt.int16, kind="ExternalInput")
    t_S = nc.dram_tensor("S", [128, NCH * W], BF16, kind="ExternalInput")
    t_efp = nc.dram_tensor("efp", [128, NCH * DE], F32, kind="ExternalInput")
    t_deg = nc.dram_tensor("deg", [128, R], BF16, kind="ExternalInput")
    t_nfp = nc.dram_tensor("nfp", [128, NROWT * 128], F32, kind="ExternalInput")
    t_UsT = nc.dram_tensor("UsT", [T, D, D], F32, kind="ExternalInput")
    t_UdT = nc.dram_tensor("UdT", [T, D, D], F32, kind="ExternalInput")
    t_UeT = nc.dram_tensor("UeT", [T, DE, D], F32, kind="ExternalInput")
    t_UbR = nc.dram_tensor("UbR", [T, 1, D], F32, kind="ExternalInput")
    t_M1T = nc.dram_tensor("M1T", [T, D, D], F32, kind="ExternalInput")
    t_M2T = nc.dram_tensor("M2T", [T, D, D], F32, kind="ExternalInput")
    t_Mb = nc.dram_tensor("Mb", [T, D, 1], F32, kind="ExternalInput")
    t_out = nc.dram_tensor("h_out", [128, NROWT * 128], F32, kind="ExternalOutput")

    with tile.TileContext(nc) as tc:
        with (
            tc.tile_pool(name="res", bufs=1) as res,
            tc.tile_pool(name="stream", bufs=3) as stp,
            tc.tile_pool(name="tmp", bufs=3) as tmp,
            tc.tile_pool(name="ps", bufs=2, space="PSUM") as ps,
            tc.tile_pool(name="dram", bufs=1, space="DRAM") as dr,
        ):
            ilo = res.tile([128, NW * C_LO * 8], mybir.dt.int16)
            nc.sync.dma_start(out=ilo[:], in_=t_idx_lo[:, :])
            ihi = res.tile([128, NW * C_HI * 8], mybir.dt.int16)
            nc.sync.dma_start(out=ihi[:], in_=t_idx_hi[:, :])
            degb = res.tile([128, R], BF16)
            nc.sync.dma_start(out=degb[:], in_=t_deg[:, :])

            def load_w(tsrc, k, P, name):
                raw = tmp.tile([P, D], F32, name=f"{name}raw{k}", tag="wraw")
                nc.sync.dma_start(out=raw[:], in_=tsrc[k, :, :])
                r = res.tile([P, D], F32R, name=f"{name}{k}", tag=f"{name}{k}")
                nc.vector.tensor_copy(out=r[:], in_=raw[:])
                return r

            UsT = [load_w(t_UsT, k, D, "UsT") for k in range(T)]
            UdT = [load_w(t_UdT, k, D, "UdT") for k in range(T)]
            UeT = [load_w(t_UeT, k, DE, "UeT") for k in range(T)]
            M1T = [load_w(t_M1T, k, D, "M1T") for k in range(T)]
            M2T = [load_w(t_M2T, k, D, "M2T") for k in range(T)]
            UbR = [load_w(t_UbR, k, 1, "UbR") for k in range(T)] if use_ub else None
            Mb = []
            for k in range(T):
                mb_r = res.tile([D, 1], F32, name=f"Mb{k}", tag=f"Mb{k}")
                nc.sync.dma_start(out=mb_r[:], in_=t_Mb[k, :, :])
                Mb.append(mb_r)
            identf = res.tile([128, 128], F32)
            make_identity(nc, identf[:])
            identr = res.tile([128, 128], F32R)
            nc.vector.tensor_copy(out=identr[:], in_=identf[:])

            hT = [res.tile([128, R], F32R, name=f"hT{i}", tag=f"hT{i}") for i in range(2)]
            aggT = res.tile([128, R], F32R)
            eaggT = res.tile([DE, R], F32R)

            # PRE: node_feat row tiles -> transpose -> hT[0]
            for t in range(NROWT):
                raw = tmp.tile([128, 128], F32, tag="nfraw")
                nc.sync.dma_start(out=raw[:], in_=t_nfp[:, t * 128:(t + 1) * 128])
                pp = ps.tile([128, 128], F32, tag="rows", space="PSUM")
                nc.tensor.transpose(out=pp[:], in_=raw[:], identity=identf[:])
                nc.vector.tensor_copy(out=hT[0][:, t * 128:(t + 1) * 128], in_=pp[:])

            ag_in = [dr.tile([R, D], BF16, name=f"agin{k}", tag=f"agin{k}") for k in range(T)]
            hud = [
                dr.tile([NC * R, D], BF16, name=f"hud{k}", tag=f"hud{k}", addr_space="Shared")
                for k in range(T)
            ]

            def emit_rows(hsrc, wT, dst_dram, out_dtype, dst_is_out):
                for t in range(NROWT):
                    pp = ps.tile([128, 128], F32, tag="rows", space="PSUM")
                    nc.tensor.matmul(
                        out=pp[:], lhsT=hsrc[:, t * 128:(t + 1) * 128], rhs=wT[:],
                        start=True, stop=True,
                    )
                    rs = tmp.tile([128, 128], out_dtype, tag="rowsb" if out_dtype == BF16 else "rowso")
                    nc.vector.tensor_copy(out=rs[:], in_=pp[:])
                    if dst_is_out:
                        nc.sync.dma_start(out=dst_dram[:, t * 128:(t + 1) * 128], in_=rs[:])
                    else:
                        nc.sync.dma_start(out=dst_dram[t * 128:(t + 1) * 128, :], in_=rs[:])

            def allgather(k):
                nc.gpsimd.collective_compute(
                    "AllGather", mybir.AluOpType.bypass,
                    replica_groups=[list(range(NC))],
                    ins=[ag_in[k][:].opt()], outs=[hud[k][:].opt()],
                )

            emit_rows(hT[0], UdT[0][:], ag_in[0][:, :], BF16, False)
            allgather(0)

            def seg_phase(round_k, is_eagg):
                P_out = DE if is_eagg else 128
                dest = eaggT if is_eagg else aggT
                for t in range(NT):
                    w0 = t * WPT
                    w1 = min(w0 + WPT, NW)
                    cols = (w1 - w0) * W
                    pt = ps.tile([128, 512], F32, tag="agg", space="PSUM")
                    first = True
                    if not is_eagg:
                        hd = tmp.tile([128, 512], F32R, tag="hdeg")
                        nc.vector.tensor_tensor(
                            out=hd[:, :cols],
                            in0=hT[round_k % 2][:, w0 * W: w0 * W + cols],
                            in1=degb[:, w0 * W: w0 * W + cols],
                            op=mybir.AluOpType.mult,
                        )
                        nc.tensor.matmul(out=pt[:, :cols], lhsT=UsT[round_k][:],
                                         rhs=hd[:, :cols], start=True, stop=False,
                                         skip_group_check=True)
                        nc.tensor.matmul(out=pt[:, :cols], lhsT=UeT[round_k][:],
                                         rhs=eaggT[:, w0 * W: w0 * W + cols],
                                         start=False, stop=False, skip_group_check=True)
                        if use_ub:
                            nc.tensor.matmul(out=pt[:, :cols], lhsT=UbR[round_k][:],
                                             rhs=degb[0:1, w0 * W: w0 * W + cols].bitcast(BF16),
                                             start=False, stop=False, skip_group_check=True)
                        first = False
                    for w in range(w0, w1):
                        colblk = (w - w0) * W
                        Sw = stp.tile([128, CPW * W], BF16, tag="Ssb")
                        nc.sync.dma_start(out=Sw[:], in_=t_S[:, w * CPW * W:(w + 1) * CPW * W])
                        if is_eagg:
                            gEF = stp.tile([128, CPW * DE], BF16, tag="EF")
                            nc.gpsimd.dma_start(
                                out=gEF[:], in_=t_efp[:, w * CPW * DE:(w + 1) * CPW * DE])
                        else:
                            glo = stp.tile([128, C_LO * D], BF16, tag="Glo")
                            nc.gpsimd.dma_gather(
                                out_ap=glo[:].rearrange("p (k d) -> p k d", d=D),
                                in_ap=hud[round_k][0:SPLIT, :],
                                idxs_ap=ilo[:, w * C_LO * 8:(w + 1) * C_LO * 8],
                                num_idxs=C_LO * 128, num_idxs_reg=C_LO * 128,
                                elem_size=D, queue_num=0,
                            )
                            ghi = stp.tile([128, C_HI * D], BF16, tag="Ghi")
                            nc.gpsimd.dma_gather(
                                out_ap=ghi[:].rearrange("p (k d) -> p k d", d=D),
                                in_ap=hud[round_k][SPLIT:NC * R, :],
                                idxs_ap=ihi[:, w * C_HI * 8:(w + 1) * C_HI * 8],
                                num_idxs=C_HI * 128, num_idxs_reg=C_HI * 128,
                                elem_size=D, queue_num=0,
                            )
                        for cc in range(CPW):
                            last = (w == w1 - 1) and (cc == CPW - 1)
                            if is_eagg:
                                lhs = gEF[:, cc * DE:(cc + 1) * DE]
                            elif cc < C_LO:
                                lhs = glo[:, cc * D:(cc + 1) * D]
                            else:
                                lhs = ghi[:, (cc - C_LO) * D:(cc - C_LO + 1) * D]
                            nc.tensor.matmul(
                                out=pt[:P_out, colblk:colblk + W],
                                lhsT=lhs,
                                rhs=Sw[:, cc * W:(cc + 1) * W],
                                start=first, stop=last, skip_group_check=True,
                            )
                            first = False
                    nc.vector.tensor_copy(out=dest[:P_out, w0 * W: w0 * W + cols],
                                          in_=pt[:P_out, :cols])
                    if not is_eagg:
                        pu = ps.tile([128, 512], F32, tag="upd", space="PSUM")
                        nc.tensor.matmul(out=pu[:, :cols], lhsT=M1T[round_k][:],
                                         rhs=hT[round_k % 2][:, w0 * W: w0 * W + cols],
                                         start=True, stop=False, skip_group_check=True)
                        nc.tensor.matmul(out=pu[:, :cols], lhsT=M2T[round_k][:],
                                         rhs=aggT[:, w0 * W: w0 * W + cols],
                                         start=False, stop=True, skip_group_check=True)
                        nc.scalar.activation(
                            out=hT[(round_k + 1) % 2][:, w0 * W: w0 * W + cols],
                            in_=pu[:, :cols],
                            func=mybir.ActivationFunctionType.Relu,
                            bias=Mb[round_k][:],
                        )

            seg_phase(0, True)          # E_agg precompute
            for k in range(T):
                seg_phase(k, False)
                if k < T - 1:
                    emit_rows(hT[(k + 1) % 2], UdT[k + 1][:], ag_in[k + 1][:, :], BF16, False)
                    allgather(k + 1)
                else:
                    emit_rows(hT[(k + 1) % 2], identr[:], t_out, F32, True)

    nc.compile()
    res_k = run_bass_kernel_spmd(nc, in_maps, core_ids=list(range(NC)))
    h = np.zeros((NC * R, D), np.float32)
    for c in range(NC):
        o = res_k.results[c]["h_out"]
        for t in range(NROWT):
            h[c * R + t * 128: c * R + (t + 1) * 128, :] = o[:, t * 128:(t + 1) * 128]
    return edge_feat, h[:N_NODES]
